# revision 1
# baseline (speedup 1.0000x reference)
"""DiT attention (B=2, T=2048, D=1024, H=16, rope on head 0) on 8 trn2 cores.

Sharding: tensor-parallel over heads. Core c owns heads {2c, 2c+1}:
  - QKV projection: column-sharded (384 features per core), x^T replicated.
  - Attention: fully local per (batch, head); computed transposed
    (S^T = K^T' @ Q^T per 128-key tile) so softmax's exp evicts PSUM->SBUF
    on the ACT engine; row-sums come free via an appended ones-column on V
    (out^T psum rows 0-63 = head out, row 64 = softmax denominator).
  - Out projection: row-sharded; per-core partial [4096, 1024] summed on host.
All matmuls run as float32r (full PE rate at N=512).
"""
import sys, os
sys.path.insert(0, "/opt/trn_rl_repo")
import numpy as np

B, T, D, H, HD = 2, 2048, 1024, 16, 64
NCORES = 8
NTOK = B * T            # 4096
TT = 4                  # token tiles of 512 per batch (projection)
KC = 8                  # contraction chunks of 128 over D
NKT = T // 128          # 16 key tiles
QC = 4                  # q chunks of 512 per batch
ROPE_BASE = 10000.0
REPEATS = 1  # >1: repeat the whole computation on-device (timing differential)

_CACHE = {}


def _build():
    import concourse.bacc as bacc
    import concourse.mybir as mybir
    import concourse.tile as tile

    F32 = mybir.dt.float32
    F32R = mybir.dt.float32r
    EXP = mybir.ActivationFunctionType.Exp

    nc = bacc.Bacc("TRN2", target_bir_lowering=False, debug=False, num_devices=NCORES)

    xT = nc.dram_tensor("xT", [D, NTOK], F32R, kind="ExternalInput")
    wqkv = nc.dram_tensor("wqkv", [D, 384], F32R, kind="ExternalInput")
    wout = nc.dram_tensor("wout", [128, D], F32R, kind="ExternalInput")
    cosT = nc.dram_tensor("cosT", [64, T], F32R, kind="ExternalInput")
    sinT = nc.dram_tensor("sinT", [64, T], F32R, kind="ExternalInput")
    maskb = nc.dram_tensor("maskb", [128, B * NKT], F32, kind="ExternalInput")
    ident = nc.dram_tensor("ident", [128, 128], F32, kind="ExternalInput")
    out_d = nc.dram_tensor("out", [NTOK, D], F32, kind="ExternalOutput")

    with tile.TileContext(nc) as tc:
        with (
            tc.tile_pool(name="consts", bufs=1) as consts,
            tc.tile_pool(name="resid", bufs=1) as resid,
            tc.tile_pool(name="vaugp", bufs=2) as vaugp,
            tc.tile_pool(name="xtp", bufs=16) as xtp,
            tc.tile_pool(name="ptp", bufs=5) as ptp,
            tc.tile_pool(name="outst", bufs=2) as outstp,
            tc.tile_pool(name="smallp", bufs=4) as smallp,
            tc.tile_pool(name="rotp", bufs=2) as rotp,
            tc.tile_pool(name="dramp", bufs=4, space="DRAM") as dramp,
            tc.tile_pool(name="ps_sc", bufs=2, space="PSUM") as ps_sc,
            tc.tile_pool(name="ps_av", bufs=1, space="PSUM") as ps_av,
            tc.tile_pool(name="ps_sm", bufs=2, space="PSUM") as ps_sm,
        ):
            # ---- constants ----
            wq_sb = []
            for kc in range(KC):
                wt = consts.tile([128, 384], F32R, name=f"wq{kc}")
                nc.sync.dma_start(wt[:], wqkv[kc * 128:(kc + 1) * 128, :])
                wq_sb.append(wt)
            wout_sb = consts.tile([128, D], F32R)
            nc.sync.dma_start(wout_sb[:], wout[:])
            cos_sb = consts.tile([64, T], F32R)
            nc.sync.dma_start(cos_sb[:], cosT[:])
            sin_sb = consts.tile([64, T], F32R)
            nc.sync.dma_start(sin_sb[:], sinT[:])
            mb_sb = consts.tile([128, B * NKT], F32)
            nc.sync.dma_start(mb_sb[:], maskb[:])
            id_sb = consts.tile([128, 128], F32)
            nc.sync.dma_start(id_sb[:], ident[:])

            # ---- resident per-batch tensors ----
            qt_sb = [resid.tile([128, T], F32R, name=f"qt{b}") for b in range(B)]
            kt_sb = [resid.tile([128, T], F32R, name=f"kt{b}") for b in range(B)]
            # V natural layout per (b, head): [k 128, 65] tiles (col 64 = ones)
            vnat = [[resid.tile([128, NKT * 65], F32R, name=f"vn{b}{h}") for h in range(2)]
                    for b in range(B)]

            def proj_gen(b):
                """QKV projection for batch b; yields between small work items
                so it can be interleaved into the previous batch's attention.
                Transposes + RoPE are pipelined per token-tile."""
                vaug = [vaugp.tile([65, T], F32, name=f"va{b}{h}", tag=f"vaug{h}")
                        for h in range(2)]
                for h in range(2):
                    nc.gpsimd.memset(vaug[h][64:65, :], 1.0)
                for tt in range(TT):
                    g0 = b * T + tt * 512
                    sl = slice(tt * 512, (tt + 1) * 512)
                    xts = []
                    for kc in range(KC):
                        xt_t = xtp.tile([128, 512], F32R, name=f"xt{b}{tt}{kc}", tag="xt")
                        # split each tile across two HWDGE queues
                        nc.sync.dma_start(xt_t[0:64, :], xT[kc * 128:kc * 128 + 64, g0:g0 + 512])
                        nc.sync.dma_start(xt_t[64:128, :], xT[kc * 128 + 64:(kc + 1) * 128, g0:g0 + 512])
                        xts.append(xt_t)
                    for ft in range(3):  # 0=Q, 1=K, 2=V
                        ps = ps_sm.tile([128, 512], F32, name=f"pp{b}{tt}{ft}", tag="sm")
                        for kc in range(KC):
                            nc.tensor.matmul(
                                ps[:], wq_sb[kc][:, ft * 128:(ft + 1) * 128], xts[kc][:],
                                start=(kc == 0), stop=(kc == KC - 1),
                            )
                        if ft == 0:
                            nc.vector.tensor_copy(qt_sb[b][:, sl], ps[:])
                        elif ft == 1:
                            nc.vector.tensor_copy(kt_sb[b][:, sl], ps[:])
                        else:
                            nc.vector.tensor_copy(vaug[0][0:64, sl], ps[0:64, :])
                            nc.vector.tensor_copy(vaug[1][0:64, sl], ps[64:128, :])
                        yield
                    # V_aug^T -> V_nat via PE transpose (this token-tile's 4 key tiles)
                    for h in range(2):
                        for kt in range(tt * 4, (tt + 1) * 4):
                            tp = ps_sm.tile([128, 65], F32, name=f"tp{b}{h}{kt}", tag="sm")
                            nc.tensor.transpose(tp[:], vaug[h][:, kt * 128:(kt + 1) * 128],
                                                id_sb[0:65, 0:65])
                            nc.vector.tensor_copy(vnat[b][h][:, kt * 65:(kt + 1) * 65], tp[:])
                            yield
                    # RoPE on head-even rows (0:64) of this token-tile's Q^T/K^T
                    # (identity data on cores != 0)
                    for t_sb in (qt_sb[b], kt_sb[b]):
                        rot = rotp.tile([64, 512], F32R, name=f"rot{b}{tt}", tag="rot")
                        nc.gpsimd.tensor_copy(rot[0:32, :], t_sb[32:64, sl])
                        nc.gpsimd.tensor_copy(rot[32:64, :], t_sb[0:32, sl])
                        yield
                        nc.vector.tensor_mul(rot[:], rot[:], sin_sb[:, sl])
                        nc.vector.tensor_mul(t_sb[0:64, sl], t_sb[0:64, sl], cos_sb[:, sl])
                        yield
                        nc.vector.tensor_add(t_sb[0:64, sl], t_sb[0:64, sl], rot[:])
                        yield

            def pull(bgs):
                while bgs:
                    try:
                        next(bgs[0])
                        return
                    except StopIteration:
                        bgs.pop(0)

            def attn_unit(b, qc, bgs):
                """Attention for (batch b, 512-query chunk qc); PE stream is
                software-pipelined (scores one key-tile ahead of AV).
                bgs: list of background generators to interleave (next batch's
                projection, previous chunk's normalize+out-proj tail).
                Returns this chunk's tail generator."""
                q0 = qc * 512
                av = [ps_av.tile([65, 512], F32, name=f"av{b}{qc}{h}", tag=f"av{h}")
                      for h in range(2)]
                sc_t = {}

                def trace_scores(kt):
                    sc = ps_sc.tile([128, 1024], F32, name=f"sc{b}{qc}{kt}", tag="sc")
                    for h in range(2):
                        nc.tensor.matmul(
                            sc[:, h * 512:(h + 1) * 512],
                            kt_sb[b][h * 64:(h + 1) * 64, kt * 128:(kt + 1) * 128],
                            qt_sb[b][h * 64:(h + 1) * 64, q0:q0 + 512],
                            start=True, stop=True,
                        )
                    sc_t[kt] = sc

                trace_scores(0)
                for kt in range(NKT):
                    if kt + 1 < NKT:
                        trace_scores(kt + 1)
                    pt = ptp.tile([128, 1024], F32R, name=f"pt{b}{qc}{kt}", tag="pt")
                    nc.scalar.activation(pt[:], sc_t.pop(kt)[:], EXP,
                                         bias=mb_sb[:, b * NKT + kt: b * NKT + kt + 1],
                                         scale=float(HD) ** -0.5)
                    for h in range(2):
                        nc.tensor.matmul(
                            av[h][:], vnat[b][h][:, kt * 65:(kt + 1) * 65],
                            pt[:, h * 512:(h + 1) * 512],
                            start=(kt == 0), stop=(kt == NKT - 1),
                        )
                    pull(bgs)

                # Evict AV psum to SBUF right away (frees the psum bank for the
                # next chunk's accumulation) and kick off the denominator
                # broadcast chain; the actual normalization is deferred.
                av_sb, bcasts = [], []
                for h in range(2):
                    srecip = smallp.tile([1, 512], F32, name=f"sr{b}{qc}{h}", tag="sr")
                    nc.vector.reciprocal(srecip[:], av[h][64:65, :])
                    avc = smallp.tile([64, 512], F32, name=f"avc{b}{qc}{h}", tag="avc")
                    nc.vector.tensor_copy(avc[:], av[h][0:64, :])
                    av_sb.append(avc)
                    srd = dramp.tile([1, 512], F32, name=f"srd{b}{qc}{h}", tag="srd")
                    nc.sync.dma_start(srd[:], srecip[:])
                    bcast = smallp.tile([64, 512], F32, name=f"bc{b}{qc}{h}", tag="bc")
                    nc.gpsimd.dma_start(bcast[:], srd[:].broadcast_to([64, 512]))
                    bcasts.append(bcast)

                def tail():
                    # normalize (deferred softmax division) -> stacked [128 d, 512 q]
                    out_st = outstp.tile([128, 512], F32R, name=f"os{b}{qc}", tag="os")
                    for h in range(2):
                        nc.vector.tensor_mul(out_st[h * 64:(h + 1) * 64, :],
                                             av_sb[h][:], bcasts[h][:])
                        yield
                    # out projection: partial = out_st.T @ wout_slice -> DRAM
                    for qt in range(4):
                        g = b * T + q0 + qt * 128
                        for nt in range(2):
                            po = ps_sm.tile([128, 512], F32, name=f"po{b}{qc}{qt}{nt}", tag="sm")
                            nc.tensor.matmul(
                                po[:], out_st[:, qt * 128:(qt + 1) * 128],
                                wout_sb[:, nt * 512:(nt + 1) * 512],
                                start=True, stop=True,
                            )
                            ob = smallp.tile([128, 512], F32, name=f"ob{b}{qc}{qt}{nt}", tag="ob")
                            nc.vector.tensor_copy(ob[:], po[:])
                            nc.sync.dma_start(out_d[g:g + 128, nt * 512:(nt + 1) * 512], ob[:])
                            yield

                return tail()

            # ---- schedule ----
            for _rep in range(REPEATS):
                for b in range(B):
                    for _ in proj_gen(b):
                        pass
                bgs = []
                for b in range(B):
                    for qc in range(QC):
                        t = attn_unit(b, qc, bgs)
                        bgs.append(t)
                for g in bgs:
                    for _ in g:
                        pass

    nc.compile()
    return nc


def _host_inputs(x, w_qkv, w_out, mask):
    x = np.asarray(x, dtype=np.float32)
    w_qkv = np.asarray(w_qkv, dtype=np.float32)
    w_out = np.asarray(w_out, dtype=np.float32)
    mask = np.asarray(mask)

    xT = np.ascontiguousarray(x.reshape(NTOK, D).T)

    inv_freq = 1.0 / (ROPE_BASE ** (np.arange(0, HD, 2, dtype=np.float32) / HD))
    t = np.arange(T, dtype=np.float32)
    freqs = np.outer(t, inv_freq)                    # [T, 32]
    cos_r = np.cos(np.concatenate([freqs, freqs], 1)).T.astype(np.float32)  # [64, T]
    sin_half = np.sin(freqs).T.astype(np.float32)    # [32, T]
    sin_r = np.concatenate([-sin_half, sin_half], 0)  # [64, T] signed

    mb = np.zeros((128, B * NKT), dtype=np.float32)
    for b in range(B):
        for kt in range(NKT):
            mb[:, b * NKT + kt] = np.where(mask[b, kt * 128:(kt + 1) * 128], 0.0, -1e30)

    ident = np.eye(128, dtype=np.float32)

    in_maps = []
    for c in range(NCORES):
        cs = slice(c * 128, (c + 1) * 128)
        wq_c = np.ascontiguousarray(np.concatenate(
            [w_qkv[:, 0:D][:, cs], w_qkv[:, D:2 * D][:, cs], w_qkv[:, 2 * D:3 * D][:, cs]], axis=1))
        if c == 0:
            cosc, sinc = cos_r, sin_r
        else:
            cosc = np.ones_like(cos_r)
            sinc = np.zeros_like(sin_r)
        in_maps.append({
            "xT": xT,
            "wqkv": wq_c,
            "wout": np.ascontiguousarray(w_out[cs, :]),
            "cosT": cosc,
            "sinT": sinc,
            "maskb": mb,
            "ident": ident,
        })
    return in_maps


def kernel(x, w_qkv, w_out, mask):
    if "nc" not in _CACHE:
        _CACHE["nc"] = _build()
    nc = _CACHE["nc"]
    in_maps = _host_inputs(x, w_qkv, w_out, mask)

    from concourse.bass_utils import run_bass_kernel_spmd
    res = run_bass_kernel_spmd(nc, in_maps, core_ids=list(range(NCORES)))
    _CACHE["last_results"] = res

    total = np.zeros((NTOK, D), dtype=np.float32)
    for c in range(NCORES):
        total += res.results[c]["out"]
    return total.reshape(B, T, D)



# revision 3
# speedup vs baseline: 1.1193x; 1.1193x over previous
"""DiT attention (B=2, T=2048, D=1024, H=16, rope on head 0) on 8 trn2 cores.

Sharding: tensor-parallel over heads. Core c owns heads {2c, 2c+1}:
  - QKV projection: column-sharded (384 features per core), x^T replicated
    (pre-tiled on host, bf16, one resident SBUF tile per 512-token group).
  - Q^T/K^T kept transposed [dims, tokens]; V projected directly in natural
    [tokens, dims] layout (x^T tile as the stationary matmul operand), with a
    resident ones-column per key tile for the softmax denominator.
  - Attention fully local per (batch, head); scores computed per 128-key tile
    (S^T = K^T' @ Q^T), exp evicts PSUM->SBUF bf16 on the ACT engine; row
    sums ride along as psum row 64 of the AV accumulation.
  - Softmax denominator: DVE reciprocal + gpsimd partition_broadcast (no DMA).
  - Out projection row-sharded; per-core partial written bf16, summed on host.
Everything on the PE runs bf16 (1 cycle/row); the whole schedule is one
software-pipelined stream: each batch's QKV projection is interleaved into
the previous attention work so the PE fills exp-latency gaps and the ACT
engine (exp, the second-busiest floor) never starves.
"""
import sys
sys.path.insert(0, "/opt/trn_rl_repo")
import numpy as np

B, T, D, H, HD = 2, 2048, 1024, 16, 64
NCORES = 8
NTOK = B * T            # 4096
NG = 8                  # 512-token groups (b*4 + tt)
KC = 8                  # contraction chunks of 128 over D
NKT = T // 128          # 16 key tiles per batch
QC = 4                  # 512-query chunks per batch
ROPE_BASE = 10000.0

_CACHE = {}


def _build():
    import concourse.bacc as bacc
    import concourse.mybir as mybir
    import concourse.tile as tile

    F32 = mybir.dt.float32
    BF16 = mybir.dt.bfloat16
    EXP = mybir.ActivationFunctionType.Exp

    nc = bacc.Bacc("TRN2", target_bir_lowering=False, debug=False, num_devices=NCORES)

    xt_d = nc.dram_tensor("xt", [128, NG * KC * 512], BF16, kind="ExternalInput")
    wqkv = nc.dram_tensor("wqkv", [128, KC * 384], BF16, kind="ExternalInput")
    wout = nc.dram_tensor("wout", [128, D], BF16, kind="ExternalInput")
    cosT = nc.dram_tensor("cosT", [64, T], BF16, kind="ExternalInput")
    sinT = nc.dram_tensor("sinT", [64, T], BF16, kind="ExternalInput")
    maskb = nc.dram_tensor("maskb", [128, B * NKT], F32, kind="ExternalInput")
    out_d = nc.dram_tensor("out", [128, B * QC * 4096], BF16, kind="ExternalOutput")

    with tile.TileContext(nc) as tc:
        with (
            tc.tile_pool(name="consts", bufs=1) as consts,
            tc.tile_pool(name="resid", bufs=1) as resid,
            tc.tile_pool(name="xtp", bufs=1) as xtp,
            tc.tile_pool(name="ptp", bufs=3) as ptp,
            tc.tile_pool(name="rotp", bufs=2) as rotp,
            tc.tile_pool(name="smallp", bufs=2) as smallp,
            tc.tile_pool(name="outst", bufs=2) as outstp,
            tc.tile_pool(name="stgp", bufs=2) as stgp,
            tc.tile_pool(name="ps_sc", bufs=2, space="PSUM") as ps_sc,
            tc.tile_pool(name="ps_av", bufs=1, space="PSUM") as ps_av,
            tc.tile_pool(name="ps_pj", bufs=2, space="PSUM") as ps_pj,
        ):
            # ---- constants ----
            wq_sb = consts.tile([128, KC * 384], BF16)
            nc.sync.dma_start(wq_sb[:], wqkv[:])
            cos_sb = consts.tile([64, T], BF16)
            nc.sync.dma_start(cos_sb[:], cosT[:])
            sin_sb = consts.tile([64, T], BF16)
            nc.sync.dma_start(sin_sb[:], sinT[:])
            mb_sb = consts.tile([128, B * NKT], F32)
            nc.sync.dma_start(mb_sb[:], maskb[:])
            wout_sb = consts.tile([128, D], BF16)
            nc.sync.dma_start(wout_sb[:], wout[:])

            # ---- resident per-batch tensors ----
            qt_sb = [resid.tile([128, T], BF16, name=f"qt{b}") for b in range(B)]
            kt_sb = [resid.tile([128, T], BF16, name=f"kt{b}") for b in range(B)]
            # V natural layout: per batch [128 keys, (h,kt) blocks of 65]
            # (col 64 of each block stays 1.0 from the initial memset -> row
            # 64 of the AV psum accumulates the softmax denominator)
            vn_sb = [resid.tile([128, 2 * NKT * 65], BF16, name=f"vn{b}") for b in range(B)]
            for b in range(B):
                nc.gpsimd.memset(vn_sb[b][:], 1.0)

            # ---- x^T tiles: all resident, loaded up front (2 DMAs each) ----
            xts = []
            for g in range(NG):
                xt = xtp.tile([128, KC * 512], BF16, name=f"xt{g}")
                nc.sync.dma_start(xt[:, 0:2048], xt_d[:, g * 4096:g * 4096 + 2048])
                nc.sync.dma_start(xt[:, 2048:4096], xt_d[:, g * 4096 + 2048:(g + 1) * 4096])
                xts.append(xt)

            def pull(bgs):
                while bgs:
                    try:
                        next(bgs[0])
                        return
                    except StopIteration:
                        bgs.pop(0)

            def proj_gen(b, g):
                """QKV projection for (batch b, 512-token group g). K and Q land
                transposed [dims, tokens] (+rope on rows 0:64); V lands natural
                [tokens, dims] by using x^T as the stationary operand."""
                xt = xts[b * 4 + g]
                sl = slice(g * 512, (g + 1) * 512)
                for ft, dst in ((1, kt_sb[b]), (0, qt_sb[b])):
                    ps = ps_pj.tile([128, 512], F32, name=f"pj{b}{g}{ft}", tag="pj")
                    for kc in range(KC):
                        nc.tensor.matmul(
                            ps[:], wq_sb[:, kc * 384 + ft * 128:kc * 384 + (ft + 1) * 128],
                            xt[:, kc * 512:(kc + 1) * 512],
                            start=(kc == 0), stop=(kc == KC - 1),
                        )
                    nc.vector.tensor_copy(dst[:, sl], ps[:])
                    yield
                    # RoPE on head-even rows (identity data on cores != 0)
                    rot = rotp.tile([64, 512], BF16, name=f"rot{b}{g}{ft}", tag="rot")
                    nc.gpsimd.tensor_copy(rot[0:32, :], dst[32:64, sl])
                    nc.gpsimd.tensor_copy(rot[32:64, :], dst[0:32, sl])
                    yield
                    nc.vector.tensor_mul(rot[:], rot[:], sin_sb[:, sl])
                    nc.vector.tensor_mul(dst[0:64, sl], dst[0:64, sl], cos_sb[:, sl])
                    yield
                    nc.vector.tensor_add(dst[0:64, sl], dst[0:64, sl], rot[:])
                    yield
                for j in range(4):
                    kt = g * 4 + j
                    psv = ps_pj.tile([128, 128], F32, name=f"pv{b}{kt}", tag="pj")
                    for kc in range(KC):
                        nc.tensor.matmul(
                            psv[:], xt[:, kc * 512 + j * 128:kc * 512 + (j + 1) * 128],
                            wq_sb[:, kc * 384 + 256:kc * 384 + 384],
                            start=(kc == 0), stop=(kc == KC - 1),
                        )
                    for h in range(2):
                        nc.vector.tensor_copy(
                            vn_sb[b][:, (h * NKT + kt) * 65:(h * NKT + kt) * 65 + 64],
                            psv[:, h * 64:(h + 1) * 64])
                    yield

            def attn_unit(b, qc, bgs, gates):
                """Attention for (batch b, 512-query chunk qc). PE stream is
                software-pipelined (scores one key-tile ahead of AV); `gates`
                (qc==0 only) are this batch's projection generators, issued
                just-in-time before the first scores that need them; `bgs` are
                background generators (next batch's projection, previous
                chunks' normalize+out-proj tails) interleaved per key tile."""
                q0 = qc * 512
                av = [ps_av.tile([65, 512], F32, name=f"av{b}{qc}{h}", tag=f"av{h}")
                      for h in range(2)]
                sc_t = {}

                def trace_scores(kt):
                    sc = ps_sc.tile([128, 1024], F32, name=f"sc{b}{qc}{kt}", tag="sc")
                    for h in range(2):
                        nc.tensor.matmul(
                            sc[:, h * 512:(h + 1) * 512],
                            kt_sb[b][h * 64:(h + 1) * 64, kt * 128:(kt + 1) * 128],
                            qt_sb[b][h * 64:(h + 1) * 64, q0:q0 + 512],
                            start=True, stop=True,
                        )
                    sc_t[kt] = sc

                def ensure_gate(g):
                    if gates and g < len(gates) and gates[g] is not None:
                        for _ in gates[g]:
                            pull(bgs)
                        gates[g] = None

                ensure_gate(0)
                trace_scores(0)
                for kt in range(NKT):
                    if kt + 1 < NKT:
                        ensure_gate((kt + 1) // 4)
                        trace_scores(kt + 1)
                    pt = ptp.tile([128, 1024], BF16, name=f"pt{b}{qc}{kt}", tag="pt")
                    nc.scalar.activation(pt[:], sc_t.pop(kt)[:], EXP,
                                         bias=mb_sb[:, b * NKT + kt:b * NKT + kt + 1],
                                         scale=float(HD) ** -0.5)
                    for h in range(2):
                        nc.tensor.matmul(
                            av[h][:],
                            vn_sb[b][:, (h * NKT + kt) * 65:(h * NKT + kt + 1) * 65],
                            pt[:, h * 512:(h + 1) * 512],
                            start=(kt == 0), stop=(kt == NKT - 1),
                        )
                    pull(bgs)

                # Evict AV psum right away (frees the banks for the next
                # chunk) and kick off the denominator reciprocal+broadcast;
                # normalization and out-proj are deferred to tail().
                av_sb, bcasts = [], []
                for h in range(2):
                    srecip = smallp.tile([1, 512], F32, name=f"sr{b}{qc}{h}", tag=f"sr{h}")
                    nc.vector.reciprocal(srecip[:], av[h][64:65, :])
                    avc = smallp.tile([64, 512], F32, name=f"avc{b}{qc}{h}", tag=f"avc{h}")
                    nc.vector.tensor_copy(avc[:], av[h][0:64, :])
                    av_sb.append(avc)
                    bc = smallp.tile([64, 512], F32, name=f"bc{b}{qc}{h}", tag=f"bc{h}")
                    nc.gpsimd.partition_broadcast(bc[:], srecip[:])
                    bcasts.append(bc)

                def tail():
                    out_st = outstp.tile([128, 512], BF16, name=f"os{b}{qc}", tag="os")
                    for h in range(2):
                        nc.vector.tensor_mul(out_st[h * 64:(h + 1) * 64, :],
                                             av_sb[h][:], bcasts[h][:])
                        yield
                    stg = stgp.tile([128, 4096], BF16, name=f"stg{b}{qc}", tag="stg")
                    for qt in range(4):
                        for nt in range(2):
                            po = ps_pj.tile([128, 512], F32, name=f"po{b}{qc}{qt}{nt}", tag="pj")
                            nc.tensor.matmul(
                                po[:], out_st[:, qt * 128:(qt + 1) * 128],
                                wout_sb[:, nt * 512:(nt + 1) * 512],
                                start=True, stop=True,
                            )
                            nc.vector.tensor_copy(
                                stg[:, qt * 1024 + nt * 512:qt * 1024 + nt * 512 + 512], po[:])
                            yield
                    g2 = b * QC + qc
                    nc.sync.dma_start(out_d[:, g2 * 4096:(g2 + 1) * 4096], stg[:])
                    yield

                return tail()

            # ---- schedule: one interleaved stream ----
            projs = [[proj_gen(b, g) for g in range(4)] for b in range(B)]
            bgs = []
            for b in range(B):
                for qc in range(QC):
                    t = attn_unit(b, qc, bgs, projs[b] if qc == 0 else None)
                    bgs.append(t)
                if b + 1 < B:
                    # next batch's projection fills PE gaps during this
                    # batch's remaining (ACT-bound) attention chunks
                    bgs[0:0] = projs[b + 1]
            for g in bgs:
                for _ in g:
                    pass

    nc.compile()
    return nc


def _host_inputs(x, w_qkv, w_out, mask):
    import ml_dtypes
    bf = ml_dtypes.bfloat16
    x = np.asarray(x, dtype=np.float32)
    w_qkv = np.asarray(w_qkv, dtype=np.float32)
    w_out = np.asarray(w_out, dtype=np.float32)
    mask = np.asarray(mask)

    # x pre-tiled: xt[p, g*4096 + kc*512 + c] = x[token g*512+c, kc*128+p]
    xt = np.ascontiguousarray(
        x.reshape(NG, 512, KC, 128).transpose(3, 0, 2, 1).reshape(128, NG * KC * 512)
    ).astype(bf)

    inv_freq = 1.0 / (ROPE_BASE ** (np.arange(0, HD, 2, dtype=np.float32) / HD))
    t = np.arange(T, dtype=np.float32)
    freqs = np.outer(t, inv_freq)                    # [T, 32]
    cos_r = np.cos(np.concatenate([freqs, freqs], 1)).T.astype(np.float32)  # [64, T]
    sin_half = np.sin(freqs).T.astype(np.float32)    # [32, T]
    sin_r = np.concatenate([-sin_half, sin_half], 0)  # [64, T] signed

    mb = np.zeros((128, B * NKT), dtype=np.float32)
    for b in range(B):
        for kt in range(NKT):
            mb[:, b * NKT + kt] = np.where(mask[b, kt * 128:(kt + 1) * 128], 0.0, -1e30)

    in_maps = []
    for c in range(NCORES):
        cs = slice(c * 128, (c + 1) * 128)
        blocks = []
        for kc in range(KC):
            kcs = slice(kc * 128, (kc + 1) * 128)
            blocks.append(np.concatenate(
                [w_qkv[kcs, 0:D][:, cs], w_qkv[kcs, D:2 * D][:, cs],
                 w_qkv[kcs, 2 * D:3 * D][:, cs]], axis=1))
        wq_c = np.ascontiguousarray(np.concatenate(blocks, axis=1)).astype(bf)
        if c == 0:
            cosc, sinc = cos_r, sin_r
        else:
            cosc = np.ones_like(cos_r)
            sinc = np.zeros_like(sin_r)
        in_maps.append({
            "xt": xt,
            "wqkv": wq_c,
            "wout": np.ascontiguousarray(w_out[cs, :]).astype(bf),
            "cosT": cosc.astype(bf),
            "sinT": sinc.astype(bf),
            "maskb": mb,
        })
    return in_maps


def kernel(x, w_qkv, w_out, mask):
    if "nc" not in _CACHE:
        _CACHE["nc"] = _build()
    nc = _CACHE["nc"]
    in_maps = _host_inputs(x, w_qkv, w_out, mask)

    from concourse.bass_utils import run_bass_kernel_spmd
    res = run_bass_kernel_spmd(nc, in_maps, core_ids=list(range(NCORES)))
    _CACHE["last_results"] = res

    total = np.zeros((NTOK, D), dtype=np.float32)
    for c in range(NCORES):
        part = np.asarray(res.results[c]["out"]).astype(np.float32)
        # out[p, g2*4096 + qt*1024 + nt*512 + f] -> token g2*512+qt*128+p
        total += part.reshape(128, NG, 4, 2, 512).transpose(1, 2, 0, 3, 4).reshape(NTOK, D)
    return total.reshape(B, T, D)


# revision 8
# speedup vs baseline: 1.1380x; 1.0167x over previous
"""DiT attention (B=2, T=2048, D=1024, H=16, rope on head 0) on 8 trn2 cores.

Sharding: tensor-parallel over heads. Core c owns heads {2c, 2c+1}:
  - QKV projection: column-sharded (384 features per core), x^T replicated
    (pre-tiled on host, bf16, one resident SBUF tile per 512-token group).
  - Q^T/K^T kept transposed [dims, tokens]; V projected directly in natural
    [tokens, dims] layout (x^T tile as the stationary matmul operand), with a
    resident ones-column per key tile for the softmax denominator.
  - Attention fully local per (batch, head); scores computed per 128-key tile
    (S^T = K^T' @ Q^T), exp evicts PSUM->SBUF bf16 on the ACT engine; row
    sums ride along as psum row 64 of the AV accumulation.
  - Softmax denominator: DVE reciprocal + gpsimd partition_broadcast (no DMA).
  - Out projection row-sharded; per-core partial written bf16, summed on host.
Everything on the PE runs bf16 (1 cycle/row); the whole schedule is one
software-pipelined stream: each batch's QKV projection is interleaved into
the previous attention work so the PE fills exp-latency gaps and the ACT
engine (exp, the second-busiest floor) never starves.
"""
import sys
sys.path.insert(0, "/opt/trn_rl_repo")
import numpy as np

B, T, D, H, HD = 2, 2048, 1024, 16, 64
NCORES = 8
NTOK = B * T            # 4096
NG = 8                  # 512-token groups (b*4 + tt)
KC = 8                  # contraction chunks of 128 over D
NKT = T // 128          # 16 key tiles per batch
QC = 4                  # 512-query chunks per batch
ROPE_BASE = 10000.0

_CACHE = {}


def _build():
    import concourse.bacc as bacc
    import concourse.mybir as mybir
    import concourse.tile as tile

    F32 = mybir.dt.float32
    BF16 = mybir.dt.bfloat16
    EXP = mybir.ActivationFunctionType.Exp

    nc = bacc.Bacc("TRN2", target_bir_lowering=False, debug=False, num_devices=NCORES)

    xt_d = nc.dram_tensor("xt", [128, NG * KC * 512], BF16, kind="ExternalInput")
    wqkv = nc.dram_tensor("wqkv", [128, KC * 384], BF16, kind="ExternalInput")
    wout = nc.dram_tensor("wout", [128, D], BF16, kind="ExternalInput")
    cosT = nc.dram_tensor("cosT", [64, T], BF16, kind="ExternalInput")
    sinT = nc.dram_tensor("sinT", [64, T], BF16, kind="ExternalInput")
    maskb = nc.dram_tensor("maskb", [128, B * NKT], F32, kind="ExternalInput")
    out_d = nc.dram_tensor("out", [128, B * QC * 4096], BF16, kind="ExternalOutput")

    with tile.TileContext(nc) as tc:
        with (
            tc.tile_pool(name="consts", bufs=1) as consts,
            tc.tile_pool(name="resid", bufs=1) as resid,
            tc.tile_pool(name="xtp", bufs=1) as xtp,
            tc.tile_pool(name="ptp", bufs=3) as ptp,
            tc.tile_pool(name="rotp", bufs=2) as rotp,
            tc.tile_pool(name="smallp", bufs=2) as smallp,
            tc.tile_pool(name="outst", bufs=2) as outstp,
            tc.tile_pool(name="stgp", bufs=2) as stgp,
            tc.tile_pool(name="ps_sc", bufs=2, space="PSUM") as ps_sc,
            tc.tile_pool(name="ps_av", bufs=1, space="PSUM") as ps_av,
            tc.tile_pool(name="ps_pj", bufs=2, space="PSUM") as ps_pj,
        ):
            # ---- constants + x^T tiles, ordered to shorten the critical
            # startup chain (first scores need wq + xt0 + cos/sin) ----
            wq_sb = consts.tile([128, KC * 384], BF16)
            nc.sync.dma_start(wq_sb[:], wqkv[:])
            xts = [xtp.tile([128, KC * 512], BF16, name=f"xt{g}") for g in range(NG)]

            def load_xt(g):
                nc.sync.dma_start(xts[g][:, 0:2048], xt_d[:, g * 4096:g * 4096 + 2048])
                nc.sync.dma_start(xts[g][:, 2048:4096],
                                  xt_d[:, g * 4096 + 2048:(g + 1) * 4096])

            load_xt(0)
            cos_sb = consts.tile([64, T], BF16)
            nc.sync.dma_start(cos_sb[:], cosT[:])
            sin_sb = consts.tile([64, T], BF16)
            nc.sync.dma_start(sin_sb[:], sinT[:])
            mb_sb = consts.tile([128, B * NKT], F32)
            nc.sync.dma_start(mb_sb[:], maskb[:])
            for g in range(1, NG):
                load_xt(g)
            wout_sb = consts.tile([128, D], BF16)
            nc.sync.dma_start(wout_sb[:], wout[:])

            # ---- resident per-batch tensors ----
            qt_sb = [resid.tile([128, T], BF16, name=f"qt{b}") for b in range(B)]
            kt_sb = [resid.tile([128, T], BF16, name=f"kt{b}") for b in range(B)]
            # V natural layout: per batch [128 keys, (h,kt) blocks of 65]
            # (col 64 of each block stays 1.0 from the initial memset -> row
            # 64 of the AV psum accumulates the softmax denominator)
            vn_sb = [resid.tile([128, 2 * NKT * 65], BF16, name=f"vn{b}") for b in range(B)]
            for b in range(B):
                nc.gpsimd.memset(vn_sb[b][:], 1.0)

            def pull(bgs, want_pe=False):
                """Advance background generators by one item; with want_pe,
                keep going until an item that issued PE work (so exp-latency
                gaps in the foreground stream get matmul filler)."""
                steps = 0
                while bgs and steps < 8:
                    try:
                        tag = next(bgs[0])
                    except StopIteration:
                        bgs.pop(0)
                        continue
                    steps += 1
                    if not want_pe or tag == "pe":
                        return

            def proj_gen(b, g):
                """QKV projection for (batch b, 512-token group g). K and Q land
                transposed [dims, tokens] (+rope on rows 0:64); V lands natural
                [tokens, dims] by using x^T as the stationary operand."""
                xt = xts[b * 4 + g]
                sl = slice(g * 512, (g + 1) * 512)
                for ft, dst in ((1, kt_sb[b]), (0, qt_sb[b])):
                    ps = ps_pj.tile([128, 512], F32, name=f"pj{b}{g}{ft}", tag="pj")
                    for kc in range(KC):
                        nc.tensor.matmul(
                            ps[:], wq_sb[:, kc * 384 + ft * 128:kc * 384 + (ft + 1) * 128],
                            xt[:, kc * 512:(kc + 1) * 512],
                            start=(kc == 0), stop=(kc == KC - 1),
                        )
                    nc.vector.tensor_copy(dst[:, sl], ps[:])
                    yield "pe"
                    # RoPE on head-even rows (identity data on cores != 0)
                    rot = rotp.tile([64, 512], BF16, name=f"rot{b}{g}{ft}", tag="rot")
                    nc.gpsimd.tensor_copy(rot[0:32, :], dst[32:64, sl])
                    nc.gpsimd.tensor_copy(rot[32:64, :], dst[0:32, sl])
                    yield
                    nc.vector.tensor_mul(rot[:], rot[:], sin_sb[:, sl])
                    nc.vector.tensor_mul(dst[0:64, sl], dst[0:64, sl], cos_sb[:, sl])
                    yield
                    nc.vector.tensor_add(dst[0:64, sl], dst[0:64, sl], rot[:])
                    yield
                for j in range(4):
                    kt = g * 4 + j
                    psv = ps_pj.tile([128, 128], F32, name=f"pv{b}{kt}", tag="pj")
                    for kc in range(KC):
                        nc.tensor.matmul(
                            psv[:], xt[:, kc * 512 + j * 128:kc * 512 + (j + 1) * 128],
                            wq_sb[:, kc * 384 + 256:kc * 384 + 384],
                            start=(kc == 0), stop=(kc == KC - 1),
                        )
                    for h in range(2):
                        nc.vector.tensor_copy(
                            vn_sb[b][:, (h * NKT + kt) * 65:(h * NKT + kt) * 65 + 64],
                            psv[:, h * 64:(h + 1) * 64])
                    yield "pe"

            def attn_unit(b, qc, bgs, gates):
                """Attention for (batch b, 512-query chunk qc). PE stream is
                software-pipelined (scores one key-tile ahead of AV); `gates`
                (qc==0 only) are this batch's projection generators, issued
                just-in-time before the first scores that need them; `bgs` are
                background generators (next batch's projection, previous
                chunks' normalize+out-proj tails) interleaved per key tile."""
                q0 = qc * 512
                av = [ps_av.tile([65, 512], F32, name=f"av{b}{qc}{h}", tag=f"av{h}")
                      for h in range(2)]
                sc_t = {}

                def trace_scores(kt):
                    sc = ps_sc.tile([128, 1024], F32, name=f"sc{b}{qc}{kt}", tag="sc")
                    for h in range(2):
                        nc.tensor.matmul(
                            sc[:, h * 512:(h + 1) * 512],
                            kt_sb[b][h * 64:(h + 1) * 64, kt * 128:(kt + 1) * 128],
                            qt_sb[b][h * 64:(h + 1) * 64, q0:q0 + 512],
                            start=True, stop=True,
                        )
                    sc_t[kt] = sc

                def ensure_gate(g):
                    if gates and g < len(gates) and gates[g] is not None:
                        for _ in gates[g]:
                            pull(bgs)
                        gates[g] = None

                ensure_gate(0)
                trace_scores(0)
                for kt in range(NKT):
                    if kt + 1 < NKT:
                        ensure_gate((kt + 1) // 4)
                        trace_scores(kt + 1)
                    pt = ptp.tile([128, 1024], BF16, name=f"pt{b}{qc}{kt}", tag="pt")
                    nc.scalar.activation(pt[:], sc_t.pop(kt)[:], EXP,
                                         bias=mb_sb[:, b * NKT + kt:b * NKT + kt + 1],
                                         scale=float(HD) ** -0.5)
                    for h in range(2):
                        nc.tensor.matmul(
                            av[h][:],
                            vn_sb[b][:, (h * NKT + kt) * 65:(h * NKT + kt + 1) * 65],
                            pt[:, h * 512:(h + 1) * 512],
                            start=(kt == 0), stop=(kt == NKT - 1),
                        )
                    pull(bgs, want_pe=True)

                # Evict AV psum right away (frees the banks for the next
                # chunk) and kick off the denominator reciprocal+broadcast;
                # normalization and out-proj are deferred to tail().
                av_sb, bcasts = [], []
                for h in range(2):
                    srecip = smallp.tile([1, 512], F32, name=f"sr{b}{qc}{h}", tag=f"sr{h}")
                    nc.vector.reciprocal(srecip[:], av[h][64:65, :])
                    avc = smallp.tile([64, 512], F32, name=f"avc{b}{qc}{h}", tag=f"avc{h}")
                    nc.vector.tensor_copy(avc[:], av[h][0:64, :])
                    av_sb.append(avc)
                    bc = smallp.tile([64, 512], F32, name=f"bc{b}{qc}{h}", tag=f"bc{h}")
                    nc.gpsimd.partition_broadcast(bc[:], srecip[:])
                    bcasts.append(bc)

                def tail():
                    out_st = outstp.tile([128, 512], BF16, name=f"os{b}{qc}", tag="os")
                    for h in range(2):
                        nc.vector.tensor_mul(out_st[h * 64:(h + 1) * 64, :],
                                             av_sb[h][:], bcasts[h][:])
                        yield
                    g2 = b * QC + qc
                    for half in range(2):
                        stg = stgp.tile([128, 2048], BF16, name=f"stg{b}{qc}{half}",
                                        tag=f"stg{half}")
                        for qt in (2 * half, 2 * half + 1):
                            for nt in range(2):
                                po = ps_pj.tile([128, 512], F32,
                                                name=f"po{b}{qc}{qt}{nt}", tag="pj")
                                nc.tensor.matmul(
                                    po[:], out_st[:, qt * 128:(qt + 1) * 128],
                                    wout_sb[:, nt * 512:(nt + 1) * 512],
                                    start=True, stop=True,
                                )
                                col = (qt - 2 * half) * 1024 + nt * 512
                                nc.vector.tensor_copy(stg[:, col:col + 512], po[:])
                                yield "pe"
                        nc.sync.dma_start(
                            out_d[:, g2 * 4096 + half * 2048:g2 * 4096 + (half + 1) * 2048],
                            stg[:])
                        yield

                return tail()

            # ---- schedule: one interleaved stream ----
            projs = [[proj_gen(b, g) for g in range(4)] for b in range(B)]
            bgs = []
            for b in range(B):
                for qc in range(QC):
                    t = attn_unit(b, qc, bgs, projs[b] if qc == 0 else None)
                    bgs.append(t)
                if b + 1 < B:
                    # next batch's projection fills PE gaps during this
                    # batch's remaining (ACT-bound) attention chunks
                    bgs[0:0] = projs[b + 1]
            for g in bgs:
                for _ in g:
                    pass

    nc.compile()
    return nc


def _host_inputs(x, w_qkv, w_out, mask):
    import ml_dtypes
    bf = ml_dtypes.bfloat16
    x = np.asarray(x, dtype=np.float32)
    w_qkv = np.asarray(w_qkv, dtype=np.float32)
    w_out = np.asarray(w_out, dtype=np.float32)
    mask = np.asarray(mask)

    # x pre-tiled: xt[p, g*4096 + kc*512 + c] = x[token g*512+c, kc*128+p]
    xt = np.ascontiguousarray(
        x.reshape(NG, 512, KC, 128).transpose(3, 0, 2, 1).reshape(128, NG * KC * 512)
    ).astype(bf)

    inv_freq = 1.0 / (ROPE_BASE ** (np.arange(0, HD, 2, dtype=np.float32) / HD))
    t = np.arange(T, dtype=np.float32)
    freqs = np.outer(t, inv_freq)                    # [T, 32]
    cos_r = np.cos(np.concatenate([freqs, freqs], 1)).T.astype(np.float32)  # [64, T]
    sin_half = np.sin(freqs).T.astype(np.float32)    # [32, T]
    sin_r = np.concatenate([-sin_half, sin_half], 0)  # [64, T] signed

    mb = np.zeros((128, B * NKT), dtype=np.float32)
    for b in range(B):
        for kt in range(NKT):
            mb[:, b * NKT + kt] = np.where(mask[b, kt * 128:(kt + 1) * 128], 0.0, -1e30)

    in_maps = []
    for c in range(NCORES):
        cs = slice(c * 128, (c + 1) * 128)
        blocks = []
        for kc in range(KC):
            kcs = slice(kc * 128, (kc + 1) * 128)
            blocks.append(np.concatenate(
                [w_qkv[kcs, 0:D][:, cs], w_qkv[kcs, D:2 * D][:, cs],
                 w_qkv[kcs, 2 * D:3 * D][:, cs]], axis=1))
        wq_c = np.ascontiguousarray(np.concatenate(blocks, axis=1)).astype(bf)
        if c == 0:
            cosc, sinc = cos_r, sin_r
        else:
            cosc = np.ones_like(cos_r)
            sinc = np.zeros_like(sin_r)
        in_maps.append({
            "xt": xt,
            "wqkv": wq_c,
            "wout": np.ascontiguousarray(w_out[cs, :]).astype(bf),
            "cosT": cosc.astype(bf),
            "sinT": sinc.astype(bf),
            "maskb": mb,
        })
    return in_maps


def kernel(x, w_qkv, w_out, mask):
    if "nc" not in _CACHE:
        _CACHE["nc"] = _build()
    nc = _CACHE["nc"]
    in_maps = _host_inputs(x, w_qkv, w_out, mask)

    from concourse.bass_utils import run_bass_kernel_spmd
    res = run_bass_kernel_spmd(nc, in_maps, core_ids=list(range(NCORES)))
    _CACHE["last_results"] = res

    total = np.zeros((NTOK, D), dtype=np.float32)
    for c in range(NCORES):
        part = np.asarray(res.results[c]["out"]).astype(np.float32)
        # out[p, g2*4096 + qt*1024 + nt*512 + f] -> token g2*512+qt*128+p
        total += part.reshape(128, NG, 4, 2, 512).transpose(1, 2, 0, 3, 4).reshape(NTOK, D)
    return total.reshape(B, T, D)


# revision 11
# speedup vs baseline: 1.1819x; 1.0385x over previous
"""DiT attention (B=2, T=2048, D=1024, H=16, rope on head 0) on 8 trn2 cores.

Sharding: tensor-parallel over heads. Core c owns heads {2c, 2c+1}:
  - QKV projection: column-sharded (384 features per core), x^T replicated
    (pre-tiled on host, bf16, one resident SBUF tile per 512-token group).
  - Q^T/K^T kept transposed [dims, tokens]; V projected directly in natural
    [tokens, dims] layout (x^T tile as the stationary matmul operand), with a
    resident ones-column per key tile for the softmax denominator.
  - Attention fully local per (batch, head); scores computed per 128-key tile
    (S^T = K^T' @ Q^T), exp evicts PSUM->SBUF bf16 on the ACT engine; row
    sums ride along as psum row 64 of the AV accumulation.
  - Softmax denominator: DVE reciprocal + gpsimd partition_broadcast (no DMA).
  - Out projection row-sharded; per-core partial written bf16, summed on host.
Everything on the PE runs bf16 (1 cycle/row); the whole schedule is one
software-pipelined stream: each batch's QKV projection is interleaved into
the previous attention work so the PE fills exp-latency gaps and the ACT
engine (exp, the second-busiest floor) never starves.
"""
import sys
sys.path.insert(0, "/opt/trn_rl_repo")
import numpy as np

B, T, D, H, HD = 2, 2048, 1024, 16, 64
NCORES = 8
NTOK = B * T            # 4096
NG = 8                  # 512-token groups (b*4 + tt)
KC = 8                  # contraction chunks of 128 over D
NKT = T // 128          # 16 key tiles per batch
QC = 4                  # 512-query chunks per batch
ROPE_BASE = 10000.0

_CACHE = {}


def _build():
    import concourse.bacc as bacc
    import concourse.mybir as mybir
    import concourse.tile as tile

    F32 = mybir.dt.float32
    BF16 = mybir.dt.bfloat16
    EXP = mybir.ActivationFunctionType.Exp

    nc = bacc.Bacc("TRN2", target_bir_lowering=False, debug=False, num_devices=NCORES)

    xt_d = nc.dram_tensor("xt", [128, NG * KC * 512], BF16, kind="ExternalInput")
    wqkv = nc.dram_tensor("wqkv", [128, KC * 384], BF16, kind="ExternalInput")
    wout = nc.dram_tensor("wout", [128, D], BF16, kind="ExternalInput")
    cosT = nc.dram_tensor("cosT", [64, T], BF16, kind="ExternalInput")
    sinT = nc.dram_tensor("sinT", [64, T], BF16, kind="ExternalInput")
    maskb = nc.dram_tensor("maskb", [128, B * NKT], F32, kind="ExternalInput")
    out_d = nc.dram_tensor("out", [128, B * QC * 4096], BF16, kind="ExternalOutput")

    with tile.TileContext(nc) as tc:
        with (
            tc.tile_pool(name="consts", bufs=1) as consts,
            tc.tile_pool(name="resid", bufs=1) as resid,
            tc.tile_pool(name="xtp", bufs=1) as xtp,
            tc.tile_pool(name="ptp", bufs=3) as ptp,
            tc.tile_pool(name="rotp", bufs=2) as rotp,
            tc.tile_pool(name="smallp", bufs=2) as smallp,
            tc.tile_pool(name="outst", bufs=2) as outstp,
            tc.tile_pool(name="stgp", bufs=2) as stgp,
            tc.tile_pool(name="ps_sc", bufs=2, space="PSUM") as ps_sc,
            tc.tile_pool(name="ps_av", bufs=1, space="PSUM") as ps_av,
            tc.tile_pool(name="ps_pj", bufs=2, space="PSUM") as ps_pj,
        ):
            # ---- constants + x^T tiles, ordered to shorten the critical
            # startup chain (first scores need wq + xt0 + cos/sin) ----
            wq_sb = consts.tile([128, KC * 384], BF16)
            nc.sync.dma_start(wq_sb[:, 0:4 * 384], wqkv[:, 0:4 * 384])
            xts = [xtp.tile([128, KC * 512], BF16, name=f"xt{g}") for g in range(NG)]

            def load_xt(g):
                nc.sync.dma_start(xts[g][:, 0:2048], xt_d[:, g * 4096:g * 4096 + 2048])
                nc.sync.dma_start(xts[g][:, 2048:4096],
                                  xt_d[:, g * 4096 + 2048:(g + 1) * 4096])

            nc.sync.dma_start(xts[0][:, 0:2048], xt_d[:, 0:2048])
            nc.sync.dma_start(wq_sb[:, 4 * 384:], wqkv[:, 4 * 384:])
            nc.sync.dma_start(xts[0][:, 2048:4096], xt_d[:, 2048:4096])
            cos_sb = consts.tile([64, T], BF16)
            nc.sync.dma_start(cos_sb[:], cosT[:])
            sin_sb = consts.tile([64, T], BF16)
            nc.sync.dma_start(sin_sb[:], sinT[:])
            mb_sb = consts.tile([128, B * NKT], F32)
            nc.sync.dma_start(mb_sb[:], maskb[:])
            for g in range(1, NG):
                load_xt(g)
            wout_sb = consts.tile([128, D], BF16)
            nc.sync.dma_start(wout_sb[:], wout[:])

            # ---- resident per-batch tensors ----
            qt_sb = [resid.tile([128, T], BF16, name=f"qt{b}") for b in range(B)]
            kt_sb = [resid.tile([128, T], BF16, name=f"kt{b}") for b in range(B)]
            # V natural layout: per batch [128 keys, (h,kt) blocks of 65]
            # (col 64 of each block stays 1.0 from the initial memset -> row
            # 64 of the AV psum accumulates the softmax denominator)
            vn_sb = [resid.tile([128, 2 * NKT * 65], BF16, name=f"vn{b}") for b in range(B)]
            for b in range(B):
                nc.gpsimd.memset(vn_sb[b][:], 1.0)

            def pull(bgs, want_pe=False):
                """Advance background generators by one item; with want_pe,
                keep going until an item that issued PE work (so exp-latency
                gaps in the foreground stream get matmul filler)."""
                steps = 0
                while bgs and steps < 8:
                    try:
                        tag = next(bgs[0])
                    except StopIteration:
                        bgs.pop(0)
                        continue
                    steps += 1
                    if not want_pe or tag == "pe":
                        return

            def proj_gen(b, g):
                """QKV projection for (batch b, 512-token group g). K and Q land
                transposed [dims, tokens] (+rope on rows 0:64); V lands natural
                [tokens, dims] by using x^T as the stationary operand."""
                xt = xts[b * 4 + g]
                sl = slice(g * 512, (g + 1) * 512)
                for ft, dst in ((1, kt_sb[b]), (0, qt_sb[b])):
                    ps = ps_pj.tile([128, 512], F32, name=f"pj{b}{g}{ft}", tag="pj")
                    for kc in range(KC):
                        nc.tensor.matmul(
                            ps[:], wq_sb[:, kc * 384 + ft * 128:kc * 384 + (ft + 1) * 128],
                            xt[:, kc * 512:(kc + 1) * 512],
                            start=(kc == 0), stop=(kc == KC - 1),
                        )
                    nc.vector.tensor_copy(dst[:, sl], ps[:])
                    yield "pe"
                    # RoPE on head-even rows (identity data on cores != 0)
                    rot = rotp.tile([64, 512], BF16, name=f"rot{b}{g}{ft}", tag="rot")
                    nc.gpsimd.tensor_copy(rot[0:32, :], dst[32:64, sl])
                    nc.gpsimd.tensor_copy(rot[32:64, :], dst[0:32, sl])
                    yield
                    nc.vector.tensor_mul(rot[:], rot[:], sin_sb[:, sl])
                    nc.vector.tensor_mul(dst[0:64, sl], dst[0:64, sl], cos_sb[:, sl])
                    yield
                    nc.vector.tensor_add(dst[0:64, sl], dst[0:64, sl], rot[:])
                    yield
                psv = ps_pj.tile([128, 512], F32, name=f"pv{b}{g}", tag="pj")
                for j in range(4):
                    for kc in range(KC):
                        nc.tensor.matmul(
                            psv[:, j * 128:(j + 1) * 128],
                            xt[:, kc * 512 + j * 128:kc * 512 + (j + 1) * 128],
                            wq_sb[:, kc * 384 + 256:kc * 384 + 384],
                            start=(kc == 0), stop=(kc == KC - 1),
                        )
                    yield "pe"
                psv_r = psv[:].rearrange("p (j c) -> p j c", j=4)
                for h in range(2):
                    base = (h * NKT + g * 4) * 65
                    dst = vn_sb[b][:, base:base + 4 * 65].rearrange(
                        "p (j c) -> p j c", j=4)[:, :, 0:64]
                    nc.vector.tensor_copy(dst, psv_r[:, :, h * 64:(h + 1) * 64])
                yield

            def attn_unit(b, qc, bgs, gates):
                """Attention for (batch b, 512-query chunk qc). PE stream is
                software-pipelined (scores one key-tile ahead of AV); `gates`
                (qc==0 only) are this batch's projection generators, issued
                just-in-time before the first scores that need them; `bgs` are
                background generators (next batch's projection, previous
                chunks' normalize+out-proj tails) interleaved per key tile."""
                q0 = qc * 512
                av = [ps_av.tile([65, 512], F32, name=f"av{b}{qc}{h}", tag=f"av{h}")
                      for h in range(2)]
                sc_t = {}

                def trace_scores(kt):
                    sc = ps_sc.tile([128, 1024], F32, name=f"sc{b}{qc}{kt}", tag="sc")
                    for h in range(2):
                        nc.tensor.matmul(
                            sc[:, h * 512:(h + 1) * 512],
                            kt_sb[b][h * 64:(h + 1) * 64, kt * 128:(kt + 1) * 128],
                            qt_sb[b][h * 64:(h + 1) * 64, q0:q0 + 512],
                            start=True, stop=True,
                        )
                    sc_t[kt] = sc

                def ensure_gate(g):
                    if gates and g < len(gates) and gates[g] is not None:
                        for _ in gates[g]:
                            pull(bgs)
                        gates[g] = None

                ensure_gate(0)
                trace_scores(0)
                for kt in range(NKT):
                    if kt + 1 < NKT:
                        ensure_gate((kt + 1) // 4)
                        trace_scores(kt + 1)
                    pt = ptp.tile([128, 1024], BF16, name=f"pt{b}{qc}{kt}", tag="pt")
                    nc.scalar.activation(pt[:], sc_t.pop(kt)[:], EXP,
                                         bias=mb_sb[:, b * NKT + kt:b * NKT + kt + 1],
                                         scale=float(HD) ** -0.5)
                    for h in range(2):
                        nc.tensor.matmul(
                            av[h][:],
                            vn_sb[b][:, (h * NKT + kt) * 65:(h * NKT + kt + 1) * 65],
                            pt[:, h * 512:(h + 1) * 512],
                            start=(kt == 0), stop=(kt == NKT - 1),
                        )
                    pull(bgs, want_pe=True)

                # Evict AV psum right away (frees the banks for the next
                # chunk) and kick off the denominator reciprocal+broadcast;
                # normalization and out-proj are deferred to tail().
                av_sb, bcasts = [], []
                for h in range(2):
                    srecip = smallp.tile([1, 512], F32, name=f"sr{b}{qc}{h}", tag=f"sr{h}")
                    nc.vector.reciprocal(srecip[:], av[h][64:65, :])
                    avc = smallp.tile([64, 512], F32, name=f"avc{b}{qc}{h}", tag=f"avc{h}")
                    # ACT-side eviction: skips the (often backlogged) DVE
                    # queue so the av banks free quickly for the next chunk
                    nc.scalar.copy(avc[:], av[h][0:64, :])
                    av_sb.append(avc)
                    bc = smallp.tile([64, 512], F32, name=f"bc{b}{qc}{h}", tag=f"bc{h}")
                    nc.gpsimd.partition_broadcast(bc[:], srecip[:])
                    bcasts.append(bc)

                def tail():
                    out_st = outstp.tile([128, 512], BF16, name=f"os{b}{qc}", tag="os")
                    for h in range(2):
                        nc.vector.tensor_mul(out_st[h * 64:(h + 1) * 64, :],
                                             av_sb[h][:], bcasts[h][:])
                        yield
                    g2 = b * QC + qc
                    for half in range(2):
                        stg = stgp.tile([128, 2048], BF16, name=f"stg{b}{qc}{half}",
                                        tag=f"stg{half}")
                        for qt in (2 * half, 2 * half + 1):
                            for nt in range(2):
                                po = ps_pj.tile([128, 512], F32,
                                                name=f"po{b}{qc}{qt}{nt}", tag="pj")
                                nc.tensor.matmul(
                                    po[:], out_st[:, qt * 128:(qt + 1) * 128],
                                    wout_sb[:, nt * 512:(nt + 1) * 512],
                                    start=True, stop=True,
                                )
                                col = (qt - 2 * half) * 1024 + nt * 512
                                nc.vector.tensor_copy(stg[:, col:col + 512], po[:])
                                yield "pe"
                        nc.sync.dma_start(
                            out_d[:, g2 * 4096 + half * 2048:g2 * 4096 + (half + 1) * 2048],
                            stg[:])
                        yield

                return tail()

            # ---- schedule: one interleaved stream ----
            projs = [[proj_gen(b, g) for g in range(4)] for b in range(B)]
            bgs = []
            for b in range(B):
                for qc in range(QC):
                    t = attn_unit(b, qc, bgs, projs[b] if qc == 0 else None)
                    bgs.append(t)
                if b + 1 < B:
                    # next batch's projection fills PE gaps during this
                    # batch's remaining (ACT-bound) attention chunks
                    bgs[0:0] = projs[b + 1]
            for g in bgs:
                for _ in g:
                    pass

    nc.compile()
    return nc


def _host_inputs(x, w_qkv, w_out, mask):
    import ml_dtypes
    bf = ml_dtypes.bfloat16
    x = np.asarray(x, dtype=np.float32)
    w_qkv = np.asarray(w_qkv, dtype=np.float32)
    w_out = np.asarray(w_out, dtype=np.float32)
    mask = np.asarray(mask)

    # x pre-tiled: xt[p, g*4096 + kc*512 + c] = x[token g*512+c, kc*128+p]
    xt = np.ascontiguousarray(
        x.reshape(NG, 512, KC, 128).transpose(3, 0, 2, 1).reshape(128, NG * KC * 512)
    ).astype(bf)

    inv_freq = 1.0 / (ROPE_BASE ** (np.arange(0, HD, 2, dtype=np.float32) / HD))
    t = np.arange(T, dtype=np.float32)
    freqs = np.outer(t, inv_freq)                    # [T, 32]
    cos_r = np.cos(np.concatenate([freqs, freqs], 1)).T.astype(np.float32)  # [64, T]
    sin_half = np.sin(freqs).T.astype(np.float32)    # [32, T]
    sin_r = np.concatenate([-sin_half, sin_half], 0)  # [64, T] signed

    mb = np.zeros((128, B * NKT), dtype=np.float32)
    for b in range(B):
        for kt in range(NKT):
            mb[:, b * NKT + kt] = np.where(mask[b, kt * 128:(kt + 1) * 128], 0.0, -1e30)

    in_maps = []
    for c in range(NCORES):
        cs = slice(c * 128, (c + 1) * 128)
        blocks = []
        for kc in range(KC):
            kcs = slice(kc * 128, (kc + 1) * 128)
            blocks.append(np.concatenate(
                [w_qkv[kcs, 0:D][:, cs], w_qkv[kcs, D:2 * D][:, cs],
                 w_qkv[kcs, 2 * D:3 * D][:, cs]], axis=1))
        wq_c = np.ascontiguousarray(np.concatenate(blocks, axis=1)).astype(bf)
        if c == 0:
            cosc, sinc = cos_r, sin_r
        else:
            cosc = np.ones_like(cos_r)
            sinc = np.zeros_like(sin_r)
        in_maps.append({
            "xt": xt,
            "wqkv": wq_c,
            "wout": np.ascontiguousarray(w_out[cs, :]).astype(bf),
            "cosT": cosc.astype(bf),
            "sinT": sinc.astype(bf),
            "maskb": mb,
        })
    return in_maps


def kernel(x, w_qkv, w_out, mask):
    if "nc" not in _CACHE:
        _CACHE["nc"] = _build()
    nc = _CACHE["nc"]
    in_maps = _host_inputs(x, w_qkv, w_out, mask)

    from concourse.bass_utils import run_bass_kernel_spmd
    res = run_bass_kernel_spmd(nc, in_maps, core_ids=list(range(NCORES)))
    _CACHE["last_results"] = res

    total = np.zeros((NTOK, D), dtype=np.float32)
    for c in range(NCORES):
        part = np.asarray(res.results[c]["out"]).astype(np.float32)
        # out[p, g2*4096 + qt*1024 + nt*512 + f] -> token g2*512+qt*128+p
        total += part.reshape(128, NG, 4, 2, 512).transpose(1, 2, 0, 3, 4).reshape(NTOK, D)
    return total.reshape(B, T, D)


# revision 12
# speedup vs baseline: 1.2034x; 1.0182x over previous
"""DiT attention (B=2, T=2048, D=1024, H=16, rope on head 0) on 8 trn2 cores.

Sharding: tensor-parallel over heads. Core c owns heads {2c, 2c+1}:
  - QKV projection: column-sharded (384 features per core), x^T replicated
    (pre-tiled on host, bf16, one resident SBUF tile per 512-token group).
  - Q^T/K^T kept transposed [dims, tokens]; V projected directly in natural
    [tokens, dims] layout (x^T tile as the stationary matmul operand), with a
    resident ones-column per key tile for the softmax denominator.
  - Attention fully local per (batch, head); scores computed per 128-key tile
    (S^T = K^T' @ Q^T), exp evicts PSUM->SBUF bf16 on the ACT engine; row
    sums ride along as psum row 64 of the AV accumulation.
  - Softmax denominator: DVE reciprocal + gpsimd partition_broadcast (no DMA).
  - Out projection row-sharded; per-core partial written bf16, summed on host.
Everything on the PE runs bf16 (1 cycle/row); the whole schedule is one
software-pipelined stream: each batch's QKV projection is interleaved into
the previous attention work so the PE fills exp-latency gaps and the ACT
engine (exp, the second-busiest floor) never starves.
"""
import sys
sys.path.insert(0, "/opt/trn_rl_repo")
import numpy as np

B, T, D, H, HD = 2, 2048, 1024, 16, 64
NCORES = 8
NTOK = B * T            # 4096
NG = 8                  # 512-token groups (b*4 + tt)
KC = 8                  # contraction chunks of 128 over D
NKT = T // 128          # 16 key tiles per batch
QC = 4                  # 512-query chunks per batch
ROPE_BASE = 10000.0

_CACHE = {}


def _build():
    import concourse.bacc as bacc
    import concourse.mybir as mybir
    import concourse.tile as tile

    F32 = mybir.dt.float32
    BF16 = mybir.dt.bfloat16
    EXP = mybir.ActivationFunctionType.Exp

    nc = bacc.Bacc("TRN2", target_bir_lowering=False, debug=False, num_devices=NCORES)

    xt_d = nc.dram_tensor("xt", [128, NG * KC * 512], BF16, kind="ExternalInput")
    wqkv = nc.dram_tensor("wqkv", [128, KC * 384], BF16, kind="ExternalInput")
    wout = nc.dram_tensor("wout", [128, D], BF16, kind="ExternalInput")
    cosT = nc.dram_tensor("cosT", [64, T], BF16, kind="ExternalInput")
    sinT = nc.dram_tensor("sinT", [64, T], BF16, kind="ExternalInput")
    maskb = nc.dram_tensor("maskb", [128, B * NKT], F32, kind="ExternalInput")
    out_d = nc.dram_tensor("out", [128, B * QC * 4096], BF16, kind="ExternalOutput")

    with tile.TileContext(nc) as tc:
        with (
            tc.tile_pool(name="consts", bufs=1) as consts,
            tc.tile_pool(name="resid", bufs=1) as resid,
            tc.tile_pool(name="xtp", bufs=1) as xtp,
            tc.tile_pool(name="ptp", bufs=3) as ptp,
            tc.tile_pool(name="rotp", bufs=2) as rotp,
            tc.tile_pool(name="smallp", bufs=2) as smallp,
            tc.tile_pool(name="outst", bufs=2) as outstp,
            tc.tile_pool(name="stgp", bufs=2) as stgp,
            tc.tile_pool(name="ps_sc", bufs=2, space="PSUM") as ps_sc,
            tc.tile_pool(name="ps_av", bufs=1, space="PSUM") as ps_av,
            tc.tile_pool(name="ps_pj", bufs=2, space="PSUM") as ps_pj,
        ):
            # ---- constants + x^T tiles, ordered to shorten the critical
            # startup chain (first scores need wq + xt0 + cos/sin) ----
            wq_sb = consts.tile([128, KC * 384], BF16)
            nc.sync.dma_start(wq_sb[:, 0:4 * 384], wqkv[:, 0:4 * 384])
            xts = [xtp.tile([128, KC * 512], BF16, name=f"xt{g}") for g in range(NG)]

            def load_xt(g):
                nc.sync.dma_start(xts[g][:, 0:2048], xt_d[:, g * 4096:g * 4096 + 2048])
                nc.sync.dma_start(xts[g][:, 2048:4096],
                                  xt_d[:, g * 4096 + 2048:(g + 1) * 4096])

            nc.sync.dma_start(xts[0][:, 0:2048], xt_d[:, 0:2048])
            nc.sync.dma_start(wq_sb[:, 4 * 384:], wqkv[:, 4 * 384:])
            nc.sync.dma_start(xts[0][:, 2048:4096], xt_d[:, 2048:4096])
            cos_sb = consts.tile([64, T], BF16)
            nc.sync.dma_start(cos_sb[:], cosT[:])
            sin_sb = consts.tile([64, T], BF16)
            nc.sync.dma_start(sin_sb[:], sinT[:])
            mb_sb = consts.tile([128, B * NKT], F32)
            nc.sync.dma_start(mb_sb[:], maskb[:])
            for g in range(1, NG):
                load_xt(g)
            wout_sb = consts.tile([128, D], BF16)
            nc.sync.dma_start(wout_sb[:], wout[:])

            # ---- resident per-batch tensors ----
            qt_sb = [resid.tile([128, T], BF16, name=f"qt{b}") for b in range(B)]
            kt_sb = [resid.tile([128, T], BF16, name=f"kt{b}") for b in range(B)]
            # V natural layout: per batch [128 keys, (h,kt) blocks of 65]
            # (col 64 of each block stays 1.0 from the initial memset -> row
            # 64 of the AV psum accumulates the softmax denominator)
            vn_sb = [resid.tile([128, 2 * NKT * 65], BF16, name=f"vn{b}") for b in range(B)]
            for b in range(B):
                nc.gpsimd.memset(vn_sb[b][:], 1.0)

            def pull(bgs, want_pe=False):
                """Advance background generators by one item; with want_pe,
                keep going until an item that issued PE work (so exp-latency
                gaps in the foreground stream get matmul filler)."""
                steps = 0
                while bgs and steps < 8:
                    try:
                        tag = next(bgs[0])
                    except StopIteration:
                        bgs.pop(0)
                        continue
                    steps += 1
                    if not want_pe or tag == "pe":
                        return

            def proj_gen(b, g):
                """QKV projection for (batch b, 512-token group g). K and Q land
                transposed [dims, tokens] (+rope on rows 0:64); V lands natural
                [tokens, dims] by using x^T as the stationary operand."""
                xt = xts[b * 4 + g]
                sl = slice(g * 512, (g + 1) * 512)
                for ft, dst in ((1, kt_sb[b]), (0, qt_sb[b])):
                    ps = ps_pj.tile([128, 512], F32, name=f"pj{b}{g}{ft}", tag="pj")
                    for kc in range(KC):
                        nc.tensor.matmul(
                            ps[:], wq_sb[:, kc * 384 + ft * 128:kc * 384 + (ft + 1) * 128],
                            xt[:, kc * 512:(kc + 1) * 512],
                            start=(kc == 0), stop=(kc == KC - 1),
                        )
                    nc.vector.tensor_copy(dst[:, sl], ps[:])
                    yield "pe"
                    # RoPE on head-even rows (identity data on cores != 0)
                    rot = rotp.tile([64, 512], BF16, name=f"rot{b}{g}{ft}", tag="rot")
                    nc.gpsimd.tensor_copy(rot[0:32, :], dst[32:64, sl])
                    nc.gpsimd.tensor_copy(rot[32:64, :], dst[0:32, sl])
                    yield
                    nc.vector.tensor_mul(rot[:], rot[:], sin_sb[:, sl])
                    nc.vector.tensor_mul(dst[0:64, sl], dst[0:64, sl], cos_sb[:, sl])
                    yield
                    nc.vector.tensor_add(dst[0:64, sl], dst[0:64, sl], rot[:])
                    yield
                psv = ps_pj.tile([128, 512], F32, name=f"pv{b}{g}", tag="pj")
                for j in range(4):
                    for kc in range(KC):
                        nc.tensor.matmul(
                            psv[:, j * 128:(j + 1) * 128],
                            xt[:, kc * 512 + j * 128:kc * 512 + (j + 1) * 128],
                            wq_sb[:, kc * 384 + 256:kc * 384 + 384],
                            start=(kc == 0), stop=(kc == KC - 1),
                        )
                    yield "pe"
                psv_r = psv[:].rearrange("p (j c) -> p j c", j=4)
                for h in range(2):
                    base = (h * NKT + g * 4) * 65
                    dst = vn_sb[b][:, base:base + 4 * 65].rearrange(
                        "p (j c) -> p j c", j=4)[:, :, 0:64]
                    nc.vector.tensor_copy(dst, psv_r[:, :, h * 64:(h + 1) * 64])
                yield

            def attn_unit(b, qc, bgs, gates):
                """Attention for (batch b, 512-query chunk qc). PE stream is
                software-pipelined (scores one key-tile ahead of AV); `gates`
                (qc==0 only) are this batch's projection generators, issued
                just-in-time before the first scores that need them; `bgs` are
                background generators (next batch's projection, previous
                chunks' normalize+out-proj tails) interleaved per key tile."""
                q0 = qc * 512
                av = [ps_av.tile([65, 512], F32, name=f"av{b}{qc}{h}", tag=f"av{h}")
                      for h in range(2)]
                sc_t = {}

                def trace_scores(kt):
                    sc = ps_sc.tile([128, 1024], F32, name=f"sc{b}{qc}{kt}", tag="sc")
                    for h in range(2):
                        nc.tensor.matmul(
                            sc[:, h * 512:(h + 1) * 512],
                            kt_sb[b][h * 64:(h + 1) * 64, kt * 128:(kt + 1) * 128],
                            qt_sb[b][h * 64:(h + 1) * 64, q0:q0 + 512],
                            start=True, stop=True,
                        )
                    sc_t[kt] = sc

                def ensure_gate(g):
                    if gates and g < len(gates) and gates[g] is not None:
                        for _ in gates[g]:
                            pull(bgs)
                        gates[g] = None

                ensure_gate(0)
                trace_scores(0)
                for kt in range(NKT):
                    if kt + 1 < NKT:
                        ensure_gate((kt + 1) // 4)
                        trace_scores(kt + 1)
                    pt = ptp.tile([128, 1024], BF16, name=f"pt{b}{qc}{kt}", tag="pt")
                    nc.scalar.activation(pt[:], sc_t.pop(kt)[:], EXP,
                                         bias=mb_sb[:, b * NKT + kt:b * NKT + kt + 1],
                                         scale=float(HD) ** -0.5)
                    for h in range(2):
                        nc.tensor.matmul(
                            av[h][:],
                            vn_sb[b][:, (h * NKT + kt) * 65:(h * NKT + kt + 1) * 65],
                            pt[:, h * 512:(h + 1) * 512],
                            start=(kt == 0), stop=(kt == NKT - 1),
                        )
                    pull(bgs, want_pe=True)

                # Evict AV psum right away (frees the banks for the next
                # chunk) and kick off the denominator reciprocal+broadcast;
                # normalization and out-proj are deferred to tail(). The very
                # last chunk has no successor: skip the eviction (normalize
                # straight from psum) and fan the out-proj evictions across
                # DVE+ACT so the drain is as short as possible.
                last = (b == B - 1) and (qc == QC - 1)
                av_sb, bcasts = [], []
                for h in range(2):
                    srecip = smallp.tile([1, 512], F32, name=f"sr{b}{qc}{h}", tag=f"sr{h}")
                    nc.vector.reciprocal(srecip[:], av[h][64:65, :])
                    if last:
                        av_sb.append(av[h][0:64, :])
                    else:
                        avc = smallp.tile([64, 512], F32, name=f"avc{b}{qc}{h}",
                                          tag=f"avc{h}")
                        # batch 0: ACT has slack and the DVE queue is full of
                        # projection evictions; batch 1: ACT is exp-saturated
                        (nc.scalar.copy if b == 0 else nc.vector.tensor_copy)(
                            avc[:], av[h][0:64, :])
                        av_sb.append(avc[:])
                    bc = smallp.tile([64, 512], F32, name=f"bc{b}{qc}{h}", tag=f"bc{h}")
                    nc.gpsimd.partition_broadcast(bc[:], srecip[:])
                    bcasts.append(bc)

                def tail():
                    out_st = outstp.tile([128, 512], BF16, name=f"os{b}{qc}", tag="os")
                    for h in range(2):
                        nc.vector.tensor_mul(out_st[h * 64:(h + 1) * 64, :],
                                             av_sb[h], bcasts[h][:])
                        yield
                    g2 = b * QC + qc
                    nhalf = 4 if last else 2
                    for half in range(nhalf):
                        w = 4096 // nhalf
                        stg = stgp.tile([128, w], BF16, name=f"stg{b}{qc}{half}",
                                        tag=f"stg{half % 2}")
                        for i in range(w // 512):
                            qt, nt = divmod(half * (w // 512) + i, 2)
                            po = ps_pj.tile([128, 512], F32,
                                            name=f"po{b}{qc}{qt}{nt}", tag="pj")
                            nc.tensor.matmul(
                                po[:], out_st[:, qt * 128:(qt + 1) * 128],
                                wout_sb[:, nt * 512:(nt + 1) * 512],
                                start=True, stop=True,
                            )
                            eng = nc.scalar.copy if (last and i % 2) else \
                                nc.vector.tensor_copy
                            eng(stg[:, i * 512:(i + 1) * 512], po[:])
                            yield "pe"
                        nc.sync.dma_start(
                            out_d[:, g2 * 4096 + half * w:g2 * 4096 + (half + 1) * w],
                            stg[:])
                        yield

                return tail()

            # ---- schedule: one interleaved stream ----
            projs = [[proj_gen(b, g) for g in range(4)] for b in range(B)]
            bgs = []
            for b in range(B):
                for qc in range(QC):
                    t = attn_unit(b, qc, bgs, projs[b] if qc == 0 else None)
                    bgs.append(t)
                if b + 1 < B:
                    # next batch's projection fills PE gaps during this
                    # batch's remaining (ACT-bound) attention chunks
                    bgs[0:0] = projs[b + 1]
            for g in bgs:
                for _ in g:
                    pass

    nc.compile()
    return nc


def _host_inputs(x, w_qkv, w_out, mask):
    import ml_dtypes
    bf = ml_dtypes.bfloat16
    x = np.asarray(x, dtype=np.float32)
    w_qkv = np.asarray(w_qkv, dtype=np.float32)
    w_out = np.asarray(w_out, dtype=np.float32)
    mask = np.asarray(mask)

    # x pre-tiled: xt[p, g*4096 + kc*512 + c] = x[token g*512+c, kc*128+p]
    xt = np.ascontiguousarray(
        x.reshape(NG, 512, KC, 128).transpose(3, 0, 2, 1).reshape(128, NG * KC * 512)
    ).astype(bf)

    inv_freq = 1.0 / (ROPE_BASE ** (np.arange(0, HD, 2, dtype=np.float32) / HD))
    t = np.arange(T, dtype=np.float32)
    freqs = np.outer(t, inv_freq)                    # [T, 32]
    cos_r = np.cos(np.concatenate([freqs, freqs], 1)).T.astype(np.float32)  # [64, T]
    sin_half = np.sin(freqs).T.astype(np.float32)    # [32, T]
    sin_r = np.concatenate([-sin_half, sin_half], 0)  # [64, T] signed

    mb = np.zeros((128, B * NKT), dtype=np.float32)
    for b in range(B):
        for kt in range(NKT):
            mb[:, b * NKT + kt] = np.where(mask[b, kt * 128:(kt + 1) * 128], 0.0, -1e30)

    in_maps = []
    for c in range(NCORES):
        cs = slice(c * 128, (c + 1) * 128)
        blocks = []
        for kc in range(KC):
            kcs = slice(kc * 128, (kc + 1) * 128)
            blocks.append(np.concatenate(
                [w_qkv[kcs, 0:D][:, cs], w_qkv[kcs, D:2 * D][:, cs],
                 w_qkv[kcs, 2 * D:3 * D][:, cs]], axis=1))
        wq_c = np.ascontiguousarray(np.concatenate(blocks, axis=1)).astype(bf)
        if c == 0:
            cosc, sinc = cos_r, sin_r
        else:
            cosc = np.ones_like(cos_r)
            sinc = np.zeros_like(sin_r)
        in_maps.append({
            "xt": xt,
            "wqkv": wq_c,
            "wout": np.ascontiguousarray(w_out[cs, :]).astype(bf),
            "cosT": cosc.astype(bf),
            "sinT": sinc.astype(bf),
            "maskb": mb,
        })
    return in_maps


def kernel(x, w_qkv, w_out, mask):
    if "nc" not in _CACHE:
        _CACHE["nc"] = _build()
    nc = _CACHE["nc"]
    in_maps = _host_inputs(x, w_qkv, w_out, mask)

    from concourse.bass_utils import run_bass_kernel_spmd
    res = run_bass_kernel_spmd(nc, in_maps, core_ids=list(range(NCORES)))
    _CACHE["last_results"] = res

    total = np.zeros((NTOK, D), dtype=np.float32)
    for c in range(NCORES):
        part = np.asarray(res.results[c]["out"]).astype(np.float32)
        # out[p, g2*4096 + qt*1024 + nt*512 + f] -> token g2*512+qt*128+p
        total += part.reshape(128, NG, 4, 2, 512).transpose(1, 2, 0, 3, 4).reshape(NTOK, D)
    return total.reshape(B, T, D)


# revision 17
# speedup vs baseline: 1.2075x; 1.0034x over previous
"""DiT attention (B=2, T=2048, D=1024, H=16, rope on head 0) on 8 trn2 cores.

Sharding: tensor-parallel over heads. Core c owns heads {2c, 2c+1}:
  - QKV projection: column-sharded (384 features per core), x^T replicated
    (pre-tiled on host, bf16, one resident SBUF tile per 512-token group).
  - Q^T/K^T kept transposed [dims, tokens]; V projected directly in natural
    [tokens, dims] layout (x^T tile as the stationary matmul operand), with a
    resident ones-column per key tile for the softmax denominator.
  - Attention fully local per (batch, head); scores computed per 128-key tile
    (S^T = K^T' @ Q^T), exp evicts PSUM->SBUF bf16 on the ACT engine; row
    sums ride along as psum row 64 of the AV accumulation.
  - Softmax denominator: DVE reciprocal + gpsimd partition_broadcast (no DMA).
  - Out projection row-sharded; per-core partial written bf16, summed on host.
Everything on the PE runs bf16 (1 cycle/row); the whole schedule is one
software-pipelined stream: each batch's QKV projection is interleaved into
the previous attention work so the PE fills exp-latency gaps and the ACT
engine (exp, the second-busiest floor) never starves.
"""
import sys
sys.path.insert(0, "/opt/trn_rl_repo")
import numpy as np

B, T, D, H, HD = 2, 2048, 1024, 16, 64
NCORES = 8
NTOK = B * T            # 4096
NG = 8                  # 512-token groups (b*4 + tt)
KC = 8                  # contraction chunks of 128 over D
NKT = T // 128          # 16 key tiles per batch
QC = 4                  # 512-query chunks per batch
ROPE_BASE = 10000.0

_CACHE = {}


def _build():
    import concourse.bacc as bacc
    import concourse.mybir as mybir
    import concourse.tile as tile

    F32 = mybir.dt.float32
    BF16 = mybir.dt.bfloat16
    EXP = mybir.ActivationFunctionType.Exp

    nc = bacc.Bacc("TRN2", target_bir_lowering=False, debug=False, num_devices=NCORES)

    xt_d = nc.dram_tensor("xt", [128, NG * KC * 512], BF16, kind="ExternalInput")
    wqkv = nc.dram_tensor("wqkv", [128, KC * 384], BF16, kind="ExternalInput")
    wout = nc.dram_tensor("wout", [128, D], BF16, kind="ExternalInput")
    cosT = nc.dram_tensor("cosT", [64, T], BF16, kind="ExternalInput")
    sinT = nc.dram_tensor("sinT", [64, T], BF16, kind="ExternalInput")
    maskb = nc.dram_tensor("maskb", [128, B * NKT], F32, kind="ExternalInput")
    out_d = nc.dram_tensor("out", [128, B * QC * 4096], BF16, kind="ExternalOutput")

    with tile.TileContext(nc) as tc:
        with (
            tc.tile_pool(name="consts", bufs=1) as consts,
            tc.tile_pool(name="resid", bufs=1) as resid,
            tc.tile_pool(name="xtp", bufs=1) as xtp,
            tc.tile_pool(name="ptp", bufs=3) as ptp,
            tc.tile_pool(name="rotp", bufs=2) as rotp,
            tc.tile_pool(name="smallp", bufs=2) as smallp,
            tc.tile_pool(name="outst", bufs=2) as outstp,
            tc.tile_pool(name="stgp", bufs=2) as stgp,
            tc.tile_pool(name="ps_sc", bufs=2, space="PSUM") as ps_sc,
            tc.tile_pool(name="ps_av", bufs=1, space="PSUM") as ps_av,
            tc.tile_pool(name="ps_pj", bufs=2, space="PSUM") as ps_pj,
        ):
            # ---- constants + x^T tiles, ordered to shorten the critical
            # startup chain (first scores need wq + xt0 + cos/sin) ----
            wq_sb = consts.tile([128, KC * 384], BF16)
            nc.sync.dma_start(wq_sb[:, 0:4 * 384], wqkv[:, 0:4 * 384])
            xts = [xtp.tile([128, KC * 512], BF16, name=f"xt{g}") for g in range(NG)]

            def load_xt(g):
                nc.sync.dma_start(xts[g][:, 0:2048], xt_d[:, g * 4096:g * 4096 + 2048])
                nc.sync.dma_start(xts[g][:, 2048:4096],
                                  xt_d[:, g * 4096 + 2048:(g + 1) * 4096])

            nc.sync.dma_start(xts[0][:, 0:2048], xt_d[:, 0:2048])
            nc.sync.dma_start(wq_sb[:, 4 * 384:], wqkv[:, 4 * 384:])
            nc.sync.dma_start(xts[0][:, 2048:4096], xt_d[:, 2048:4096])
            # only the first 512 cols of cos/sin block the first rope; the
            # rest can land after the next x tile
            cos_sb = consts.tile([64, T], BF16)
            nc.sync.dma_start(cos_sb[:, 0:512], cosT[:, 0:512])
            sin_sb = consts.tile([64, T], BF16)
            nc.sync.dma_start(sin_sb[:, 0:512], sinT[:, 0:512])
            mb_sb = consts.tile([128, B * NKT], F32)
            nc.sync.dma_start(mb_sb[:], maskb[:])
            load_xt(1)
            nc.sync.dma_start(cos_sb[:, 512:], cosT[:, 512:])
            nc.sync.dma_start(sin_sb[:, 512:], sinT[:, 512:])
            for g in range(2, NG):
                load_xt(g)
            wout_sb = consts.tile([128, D], BF16)
            nc.sync.dma_start(wout_sb[:], wout[:])

            # ---- resident per-batch tensors ----
            qt_sb = [resid.tile([128, T], BF16, name=f"qt{b}") for b in range(B)]
            kt_sb = [resid.tile([128, T], BF16, name=f"kt{b}") for b in range(B)]
            # V natural layout: per batch [128 keys, (h,kt) blocks of 65]
            # (col 64 of each block stays 1.0 from the initial memset -> row
            # 64 of the AV psum accumulates the softmax denominator)
            vn_sb = [resid.tile([128, 2 * NKT * 65], BF16, name=f"vn{b}") for b in range(B)]
            for b in range(B):
                nc.gpsimd.memset(vn_sb[b][:], 1.0)


            def pull(bgs, want_pe=False):
                """Advance background generators by one item; with want_pe,
                keep going until an item that issued PE work (so exp-latency
                gaps in the foreground stream get matmul filler)."""
                steps = 0
                while bgs and steps < 8:
                    try:
                        tag = next(bgs[0])
                    except StopIteration:
                        bgs.pop(0)
                        continue
                    steps += 1
                    if not want_pe or tag == "pe":
                        return

            def proj_gen(b, g):
                """QKV projection for (batch b, 512-token group g). K and Q land
                transposed [dims, tokens] (+rope on rows 0:64); V lands natural
                [tokens, dims] by using x^T as the stationary operand."""
                xt = xts[b * 4 + g]
                sl = slice(g * 512, (g + 1) * 512)
                for ft, dst in ((1, kt_sb[b]), (0, qt_sb[b])):
                    ps = ps_pj.tile([128, 512], F32, name=f"pj{b}{g}{ft}", tag="pj")
                    for kc in range(KC):
                        nc.tensor.matmul(
                            ps[:], wq_sb[:, kc * 384 + ft * 128:kc * 384 + (ft + 1) * 128],
                            xt[:, kc * 512:(kc + 1) * 512],
                            start=(kc == 0), stop=(kc == KC - 1),
                        )
                    nc.vector.tensor_copy(dst[:, sl], ps[:])
                    yield "pe"
                    # RoPE on head-even rows (identity data on cores != 0)
                    rot = rotp.tile([64, 512], BF16, name=f"rot{b}{g}{ft}", tag="rot")
                    nc.gpsimd.tensor_copy(rot[0:32, :], dst[32:64, sl])
                    nc.gpsimd.tensor_copy(rot[32:64, :], dst[0:32, sl])
                    yield
                    nc.vector.tensor_mul(rot[:], rot[:], sin_sb[:, sl])
                    nc.vector.tensor_mul(dst[0:64, sl], dst[0:64, sl], cos_sb[:, sl])
                    yield
                    nc.vector.tensor_add(dst[0:64, sl], dst[0:64, sl], rot[:])
                    yield
                psv = ps_pj.tile([128, 512], F32, name=f"pv{b}{g}", tag="pj")
                for j in range(4):
                    for kc in range(KC):
                        nc.tensor.matmul(
                            psv[:, j * 128:(j + 1) * 128],
                            xt[:, kc * 512 + j * 128:kc * 512 + (j + 1) * 128],
                            wq_sb[:, kc * 384 + 256:kc * 384 + 384],
                            start=(kc == 0), stop=(kc == KC - 1),
                        )
                    yield "pe"
                psv_r = psv[:].rearrange("p (j c) -> p j c", j=4)
                for h in range(2):
                    base = (h * NKT + g * 4) * 65
                    dst = vn_sb[b][:, base:base + 4 * 65].rearrange(
                        "p (j c) -> p j c", j=4)[:, :, 0:64]
                    nc.vector.tensor_copy(dst, psv_r[:, :, h * 64:(h + 1) * 64])
                yield

            def attn_unit(b, qc, bgs, gates):
                """Attention for (batch b, 512-query chunk qc). PE stream is
                software-pipelined (scores one key-tile ahead of AV); `gates`
                (qc==0 only) are this batch's projection generators, issued
                just-in-time before the first scores that need them; `bgs` are
                background generators (next batch's projection, previous
                chunks' normalize+out-proj tails) interleaved per key tile."""
                q0 = qc * 512
                av = [ps_av.tile([65, 512], F32, name=f"av{b}{qc}{h}", tag=f"av{h}")
                      for h in range(2)]
                sc_t = {}

                def trace_scores(kt):
                    sc = ps_sc.tile([128, 1024], F32, name=f"sc{b}{qc}{kt}", tag="sc")
                    for h in range(2):
                        nc.tensor.matmul(
                            sc[:, h * 512:(h + 1) * 512],
                            kt_sb[b][h * 64:(h + 1) * 64, kt * 128:(kt + 1) * 128],
                            qt_sb[b][h * 64:(h + 1) * 64, q0:q0 + 512],
                            start=True, stop=True,
                        )
                    sc_t[kt] = sc

                def ensure_gate(g):
                    if gates and g < len(gates) and gates[g] is not None:
                        for _ in gates[g]:
                            pull(bgs)
                        gates[g] = None

                ensure_gate(0)
                trace_scores(0)
                for kt in range(NKT):
                    if kt + 1 < NKT:
                        ensure_gate((kt + 1) // 4)
                        trace_scores(kt + 1)
                    pt = ptp.tile([128, 1024], BF16, name=f"pt{b}{qc}{kt}", tag="pt")
                    nc.scalar.activation(pt[:], sc_t.pop(kt)[:], EXP,
                                         bias=mb_sb[:, b * NKT + kt:b * NKT + kt + 1],
                                         scale=float(HD) ** -0.5)
                    for h in range(2):
                        nc.tensor.matmul(
                            av[h][:],
                            vn_sb[b][:, (h * NKT + kt) * 65:(h * NKT + kt + 1) * 65],
                            pt[:, h * 512:(h + 1) * 512],
                            start=(kt == 0), stop=(kt == NKT - 1),
                        )
                    pull(bgs, want_pe=True)

                # Evict AV psum right away (frees the banks for the next
                # chunk) and kick off the denominator reciprocal+broadcast;
                # normalization and out-proj are deferred to tail(). The very
                # last chunk has no successor: skip the eviction (normalize
                # straight from psum) and fan the out-proj evictions across
                # DVE+ACT so the drain is as short as possible.
                last = (b == B - 1) and (qc == QC - 1)
                av_sb, bcasts = [], []
                for h in range(2):
                    srecip = smallp.tile([1, 512], F32, name=f"sr{b}{qc}{h}", tag=f"sr{h}")
                    nc.vector.reciprocal(srecip[:], av[h][64:65, :])
                    if last:
                        av_sb.append(av[h][0:64, :])
                    else:
                        avc = smallp.tile([64, 512], F32, name=f"avc{b}{qc}{h}",
                                          tag=f"avc{h}")
                        # batch 0: ACT has slack and the DVE queue is full of
                        # projection evictions -> both heads on ACT. batch 1:
                        # ACT is exp-saturated -> split across DVE + ACT so
                        # the two av banks free in parallel.
                        eng = nc.scalar.copy if (b == 0 or h == 1) else \
                            nc.vector.tensor_copy
                        eng(avc[:], av[h][0:64, :])
                        av_sb.append(avc[:])
                    bc = smallp.tile([64, 512], F32, name=f"bc{b}{qc}{h}", tag=f"bc{h}")
                    nc.gpsimd.partition_broadcast(bc[:], srecip[:])
                    bcasts.append(bc)

                def tail():
                    out_st = outstp.tile([128, 512], BF16, name=f"os{b}{qc}", tag="os")
                    for h in range(2):
                        nc.vector.tensor_mul(out_st[h * 64:(h + 1) * 64, :],
                                             av_sb[h], bcasts[h][:])
                        yield
                    g2 = b * QC + qc
                    nhalf = 4 if last else 2
                    for half in range(nhalf):
                        w = 4096 // nhalf
                        stg = stgp.tile([128, w], BF16, name=f"stg{b}{qc}{half}",
                                        tag=f"stg{half % 2}")
                        for i in range(w // 512):
                            qt, nt = divmod(half * (w // 512) + i, 2)
                            po = ps_pj.tile([128, 512], F32,
                                            name=f"po{b}{qc}{qt}{nt}", tag="pj")
                            nc.tensor.matmul(
                                po[:], out_st[:, qt * 128:(qt + 1) * 128],
                                wout_sb[:, nt * 512:(nt + 1) * 512],
                                start=True, stop=True,
                            )
                            eng = nc.scalar.copy if (last and i % 2) else \
                                nc.vector.tensor_copy
                            eng(stg[:, i * 512:(i + 1) * 512], po[:])
                            yield "pe"
                        nc.sync.dma_start(
                            out_d[:, g2 * 4096 + half * w:g2 * 4096 + (half + 1) * w],
                            stg[:])
                        yield

                return tail()

            # ---- schedule: one interleaved stream ----
            projs = [[proj_gen(b, g) for g in range(4)] for b in range(B)]
            bgs = []
            for b in range(B):
                for qc in range(QC):
                    t = attn_unit(b, qc, bgs, projs[b] if qc == 0 else None)
                    bgs.append(t)
                if b + 1 < B:
                    # next batch's projection fills PE gaps during this
                    # batch's remaining (ACT-bound) attention chunks
                    bgs[0:0] = projs[b + 1]
            for g in bgs:
                for _ in g:
                    pass

    nc.compile()
    return nc


def _host_inputs(x, w_qkv, w_out, mask):
    import ml_dtypes
    bf = ml_dtypes.bfloat16
    x = np.asarray(x, dtype=np.float32)
    w_qkv = np.asarray(w_qkv, dtype=np.float32)
    w_out = np.asarray(w_out, dtype=np.float32)
    mask = np.asarray(mask)

    # x pre-tiled: xt[p, g*4096 + kc*512 + c] = x[token g*512+c, kc*128+p]
    xt = np.ascontiguousarray(
        x.reshape(NG, 512, KC, 128).transpose(3, 0, 2, 1).reshape(128, NG * KC * 512)
    ).astype(bf)

    inv_freq = 1.0 / (ROPE_BASE ** (np.arange(0, HD, 2, dtype=np.float32) / HD))
    t = np.arange(T, dtype=np.float32)
    freqs = np.outer(t, inv_freq)                    # [T, 32]
    cos_r = np.cos(np.concatenate([freqs, freqs], 1)).T.astype(np.float32)  # [64, T]
    sin_half = np.sin(freqs).T.astype(np.float32)    # [32, T]
    sin_r = np.concatenate([-sin_half, sin_half], 0)  # [64, T] signed

    mb = np.zeros((128, B * NKT), dtype=np.float32)
    for b in range(B):
        for kt in range(NKT):
            mb[:, b * NKT + kt] = np.where(mask[b, kt * 128:(kt + 1) * 128], 0.0, -1e30)

    in_maps = []
    for c in range(NCORES):
        cs = slice(c * 128, (c + 1) * 128)
        blocks = []
        for kc in range(KC):
            kcs = slice(kc * 128, (kc + 1) * 128)
            blocks.append(np.concatenate(
                [w_qkv[kcs, 0:D][:, cs], w_qkv[kcs, D:2 * D][:, cs],
                 w_qkv[kcs, 2 * D:3 * D][:, cs]], axis=1))
        wq_c = np.ascontiguousarray(np.concatenate(blocks, axis=1)).astype(bf)
        if c == 0:
            cosc, sinc = cos_r, sin_r
        else:
            cosc = np.ones_like(cos_r)
            sinc = np.zeros_like(sin_r)
        in_maps.append({
            "xt": xt,
            "wqkv": wq_c,
            "wout": np.ascontiguousarray(w_out[cs, :]).astype(bf),
            "cosT": cosc.astype(bf),
            "sinT": sinc.astype(bf),
            "maskb": mb,
        })
    return in_maps


def kernel(x, w_qkv, w_out, mask):
    if "nc" not in _CACHE:
        _CACHE["nc"] = _build()
    nc = _CACHE["nc"]
    in_maps = _host_inputs(x, w_qkv, w_out, mask)

    from concourse.bass_utils import run_bass_kernel_spmd
    res = run_bass_kernel_spmd(nc, in_maps, core_ids=list(range(NCORES)))
    _CACHE["last_results"] = res

    total = np.zeros((NTOK, D), dtype=np.float32)
    for c in range(NCORES):
        part = np.asarray(res.results[c]["out"]).astype(np.float32)
        # out[p, g2*4096 + qt*1024 + nt*512 + f] -> token g2*512+qt*128+p
        total += part.reshape(128, NG, 4, 2, 512).transpose(1, 2, 0, 3, 4).reshape(NTOK, D)
    return total.reshape(B, T, D)
